# revision 3
# baseline (speedup 1.0000x reference)
"""BatchAllTripletLoss on 8 Trainium2 NeuronCores — padded class-slot grid.

Host layout:
  - Sort batch by label; pad each class to a slot of S = max_class_size
    columns (zero embeddings in pads). 32 classes x S=24 -> 768 padded
    anchor rows = 8 blocks x R=96 rows, one block per core; each core
    scores its 96 anchors against ALL 512 real negative columns.
  - Positives of anchor row r (slot k = r//S) are window cols [S*k, S*k+S)
    of the core's own padded cols -> band extraction is nslot FIXED
    rectangle ops (compile-time APs, SPMD-uniform, no DMA round trip).
  - The contraction gets an extra 34-row chunk: rows 0-31 carry
    1000*onehot(class) on both sides (gram accumulates 1e6 for same-class
    pairs = fused label mask, so sqrt input is never negative), row 32 is
    ones against |b|^2 (fused column-norm row), row 33 holds |a|^2 for the
    w-side row. num_valid is computed on host from label counts alone.

Device:
  - PE: grams (-2*A^T B + |b|^2 + 1e6*eq); anchor-norm bias via ACT/DVE.
  - n-side: dn = Sqrt(g + |a|^2) on ACT (f32), ndn = 32 - dn -> f16 (DVE
    tensor_scalar at 2x; the 32-shift keeps f16 error ~2e-3).
  - w-side: band2 = relu(g_w + |a|^2) per rectangle (DVE, from PSUM),
    dband = Sqrt (ACT), pos = dband + mb (f32; mb = -32 valid / -3032).
  - Hot loop over S slots x 2 single-src tensor_scalar ops (relu-sum
    accum + count accum) split DVE (fp16 4x) / ACT (Relu+Sign) / Pool.
  - No device reduction: raw accum columns are DMA'd out; host reduces.
"""
import sys
sys.path.insert(0, "/opt/trn_rl_repo")

import numpy as np
from contextlib import ExitStack

import concourse.bass as bass
import concourse.tile as tile
from concourse import bacc, mybir
from concourse.bass_utils import run_bass_kernel_spmd

F32 = mybir.dt.float32
F16 = mybir.dt.float16
Alu = mybir.AluOpType
Act = mybir.ActivationFunctionType
AX = mybir.AxisListType

B = 512
K = 4            # contraction chunks of 128 (512 dims)
KE = 34          # extra chunk rows: 32 onehot + ones + |a|^2
NB = 512         # negative columns per core
NH = 256         # half of NB
N_CORES = 8
SHIFT = 32.0
OH = 1000.0      # onehot amplitude -> 1e6 mask in the gram

_cache = {}


def _build(S: int, R: int, C: int, n_act: int = 11, n_pool: int = 0,
           loop_iters: int | None = None):
    """S = band width (max class size), R = rows per core (32-aligned
    class slots), C = number of classes."""
    SLOT = 32 * ((S + 31) // 32)
    nslot = R // SLOT
    n_dve = S - n_act - n_pool
    assert C <= 32 and R % SLOT == 0

    nc = bacc.Bacc("TRN2", target_bir_lowering=False, debug=False,
                   num_devices=N_CORES)

    lhsT_d = nc.dram_tensor("lhsT", [128, K * R], F16,
                            kind="ExternalInput")
    rhsa_d = nc.dram_tensor("rhsa", [128, K * NH], F16,
                            kind="ExternalInput")
    rhsb_d = nc.dram_tensor("rhsb", [128, K * NH], F16,
                            kind="ExternalInput")
    rhsx_d = nc.dram_tensor("rhsx", [33, NB + 2 * R], F16,
                            kind="ExternalInput")
    mb_d = nc.dram_tensor("mb", [R, S], F16, kind="ExternalInput")
    out_d = nc.dram_tensor("out", [R, 2 * S + 1], F32,
                           kind="ExternalOutput")

    with tile.TileContext(nc) as tc, ExitStack() as ctx:
        pool = ctx.enter_context(tc.tile_pool(name="sbuf", bufs=2))
        spool = ctx.enter_context(tc.tile_pool(name="scr", bufs=3))
        ppool = ctx.enter_context(tc.tile_pool(name="psum", bufs=1, space="PSUM"))

        def _body():
            # ---- input DMAs ----
            lhsT_t = pool.tile([128, K * R], F16)
            rhsa_t = pool.tile([128, K, NH], F16)
            rhsb_t = pool.tile([128, K, NH], F16)
            rhsx_t = pool.tile([33, NB + 2 * R], F16)
            mb_t = pool.tile([R, S], F16)
            nc.sync.dma_start(lhsT_t[:], lhsT_d.ap())
            nc.sync.dma_start(rhsa_t[:], rhsa_d.ap())
            nc.scalar.dma_start(rhsb_t[:], rhsb_d.ap())
            lhsT = [lhsT_t[:, k * R:(k + 1) * R] for k in range(K)]
            rhs = {0: [rhsa_t[:, k, :] for k in range(K)],
                   1: [rhsb_t[:, k, :] for k in range(K)]}
            lhs5 = rhsx_t[0:33, NB:NB + R]
            lhs5_ones = rhsx_t[32:33, NB:NB + R]
            lhs6_nrm = rhsx_t[32:33, NB + R:NB + 2 * R]
            rhs5 = {0: rhsx_t[0:33, 0:NH], 1: rhsx_t[0:33, NH:NB]}

            # ---- PE warmup: ramp the tensor engine out of low p-state ----
            nc.scalar.dma_start(rhsx_t[:], rhsx_d.ap())
            wsrc = pool.tile([128, NB], F16)
            nc.gpsimd.memset(wsrc[:], 0.0)
            nc.sync.dma_start(mb_t[:], mb_d.ap())
            dumt = pool.tile([1, 2], F32)
            nc.vector.memset(dumt[:], 1.0)
            one16 = pool.tile([33, 1], F16)
            nc.vector.memset(one16[:], 1.0)
            # pin the ACT table set (sqrt_and_others has all our funcs)
            dums = pool.tile([1, 2], F32)
            nc.scalar.activation(dums[:], dumt[:], Act.Sqrt)
            warm = ppool.tile([1, NB], F32, tag="warm", name="warm")
            for w in range(7):
                nc.tensor.matmul(warm[:], wsrc[:, 0:1], wsrc[:],
                                 start=True, stop=True)

            # ---- anchor-norm column: |a|^2 row as weights x ones column ----
            nrma = ppool.tile([R, 1], F32, tag="nrma", name="nrma")
            nc.tensor.matmul(nrma[:], lhs6_nrm, one16[32:33, :],
                             start=True, stop=True)
            nrma_s = pool.tile([R, 1], F32)
            nc.vector.tensor_copy(nrma_s[:], nrma[:])

            # ---- lhs scaled (-2x) ----
            lm2 = [pool.tile([128, R], F16, tag=f"lm2{k}", name=f"lm2{k}")
                   for k in range(K)]
            for k in range(K):
                nc.vector.tensor_scalar_mul(lm2[k][:], lhsT[k], -2.0)

            # ---- w-side gram + fused-relu band rectangles ----
            g_w = ppool.tile([R, R], F32, tag="gw", name="gw")
            for k in range(K):
                nc.tensor.matmul(g_w[:], lm2[k][:], lhsT[k],
                                 start=(k == 0), stop=False)
            nc.tensor.matmul(g_w[:], lhs5_ones, lhs6_nrm,
                             start=False, stop=True)
            band2 = pool.tile([R, S], F32)
            for k in range(nslot):
                nc.vector.tensor_scalar(
                    out=band2[k * SLOT:(k + 1) * SLOT, :],
                    in0=g_w[k * SLOT:(k + 1) * SLOT,
                            k * SLOT:k * SLOT + S],
                    scalar1=nrma_s[k * SLOT:(k + 1) * SLOT, :], scalar2=0.0,
                    op0=Alu.add, op1=Alu.max)
            dband = pool.tile([R, S], F32)
            nc.scalar.activation(dband[:], band2[:], Act.Sqrt)
            pos = pool.tile([R, S], F32)
            nc.vector.tensor_tensor(out=pos[:], in0=dband[:], in1=mb_t[:],
                                    op=Alu.add)
            negpos = pool.tile([R, S], F32)
            nc.vector.tensor_scalar_mul(negpos[:], pos[:], -1.0)

            # ---- n-side grams (onehot mask + col norms fused) ----
            ndn = pool.tile([R, NB], F16)
            for h in (0, 1):
                g = ppool.tile([R, NH], F32, tag=f"gn{h}", name=f"gn{h}")
                for k in range(K):
                    nc.tensor.matmul(g[:], lm2[k][:], rhs[h][k],
                                     start=(k == 0), stop=False)
                nc.tensor.matmul(g[:], lhs5, rhs5[h],
                                 start=False, stop=True)
                dn = spool.tile([R, NH], F32, tag=f"dn{h}", name=f"dn{h}")
                nc.scalar.activation(dn[:], g[:], Act.Sqrt,
                                     bias=nrma_s[:], scale=1.0)
                nc.vector.tensor_scalar(
                    out=ndn[:, h * NH:(h + 1) * NH], in0=dn[:],
                    scalar1=-1.0, scalar2=SHIFT,
                    op0=Alu.mult, op1=Alu.add)

            # ---- hot loop: S slots x (relu-sum + count) ----
            # DVE slots j < n_dve use the max-trick:
            #   sum(max(ndn, -p)) = sum(relu(ndn+p)) - NB*p   (host-corrected)
            # because tensor_scalar's accum reduces with op1 (must be add).
            # ACT slots (the last n_act) use native Relu/Sign accumulation.
            out_t = pool.tile([R, 2 * S + 1], F32)
            acc_a = pool.tile([R, 2 * max(n_act, 1)], F32)

            nc.vector.tensor_reduce(out=out_t[:, 2 * S:2 * S + 1],
                                    in_=pos[:, 0:n_dve], axis=AX.X,
                                    op=Alu.add)
            jd = ja = 0
            for j in range(S):
                pj = pos[:, j:j + 1]
                npj = negpos[:, j:j + 1]
                if j >= n_dve:
                    s1 = ppool.tile([R, NB], F32, tag="asc",
                                    name=f"asc1_{j}", bufs=2)
                    nc.scalar.activation(s1[:], ndn[:], Act.Relu,
                                         bias=pj, scale=1.0,
                                         accum_out=acc_a[:, 2 * ja:2 * ja + 1])
                    s2 = ppool.tile([R, NB], F32, tag="asc",
                                    name=f"asc2_{j}", bufs=2)
                    nc.scalar.activation(s2[:], ndn[:], Act.Sign,
                                         bias=pj, scale=1.0,
                                         accum_out=acc_a[:, 2 * ja + 1:2 * ja + 2])
                    ja += 1
                else:
                    s1 = spool.tile([R, NB], F16, tag="dsc", name=f"dsc1_{j}")
                    nc.vector.tensor_scalar(
                        out=s1[:], in0=ndn[:], scalar1=npj, scalar2=0.0,
                        op0=Alu.max, op1=Alu.add,
                        accum_out=out_t[:, 2 * jd:2 * jd + 1])
                    s2 = spool.tile([R, NB], F16, tag="dsc", name=f"dsc2_{j}")
                    nc.vector.tensor_scalar(
                        out=s2[:], in0=ndn[:], scalar1=npj, scalar2=0.0,
                        op0=Alu.is_gt, op1=Alu.add,
                        accum_out=out_t[:, 2 * jd + 1:2 * jd + 2])
                    jd += 1
            assert ja == n_act and jd == n_dve

            if n_act:
                nc.vector.tensor_copy(
                    out_t[:, 2 * n_dve:2 * (n_dve + n_act)], acc_a[:])

            nc.sync.dma_start(out_d.ap(), out_t[:])

        if loop_iters is None:
            _body()
        else:
            with tc.For_i(0, loop_iters, 1):
                _body()

    nc.compile()
    return nc


def _ilv(a, nchunk):
    """[nchunk*128 (contraction), x] -> [128, nchunk*x] chunk-interleaved."""
    x = a.shape[1]
    return np.ascontiguousarray(
        a.reshape(nchunk, 128, x).transpose(1, 0, 2).reshape(128, nchunk * x))


def _prepare(embeddings: np.ndarray, labels: np.ndarray):
    emb = np.ascontiguousarray(np.asarray(embeddings, dtype=np.float32))
    lab = np.asarray(labels)

    perm = np.argsort(lab, kind="stable")
    e_s = emb[perm]
    lab_s = lab[perm]
    classes, starts, counts = np.unique(lab_s, return_index=True,
                                        return_counts=True)
    C = len(classes)
    S = int(counts.max())
    SLOT = 32 * ((S + 31) // 32)
    spb = -(-C // N_CORES)            # class slots per block
    R = spb * SLOT
    assert R <= 128, f"padded rows per core {R} > 128"

    cls_of_col = np.searchsorted(starts, np.arange(B), side="right") - 1

    # padded anchors: class c -> slot c, rows [SLOT*c, SLOT*c+m_c)
    nP = N_CORES * R
    eP = np.zeros((nP, B), dtype=np.float32)
    cls_of_row = np.repeat(np.arange(-(-nP // SLOT)), SLOT)[:nP]
    live_row = np.zeros((nP,), dtype=bool)
    for c in range(C):
        eP[SLOT * c:SLOT * c + counts[c]] = \
            e_s[starts[c]:starts[c] + counts[c]]
        live_row[SLOT * c:SLOT * c + counts[c]] = True
    ePT = np.ascontiguousarray(eP.T).astype(np.float16)
    e_sT = np.ascontiguousarray(e_s.T).astype(np.float16)  # [512, 512]

    sqa_all = (ePT.astype(np.float32) ** 2).sum(0)         # [768]
    sqn = (e_sT.astype(np.float32) ** 2).sum(0)            # [512]

    # rhs extra chunks: onehot rows + |b|^2 in row 32
    def rhs_chunk(cols):
        ch = np.zeros((128, len(cols)), dtype=np.float16)
        ch[cls_of_col[cols], np.arange(len(cols))] = OH
        ch[32, :] = sqn[cols].astype(np.float16)
        return ch

    rhsa = _ilv(e_sT[:, :NH], K)
    rhsb = _ilv(e_sT[:, NH:], K)
    rhs_extra = np.ascontiguousarray(rhs_chunk(np.arange(NB))[0:33])

    num_valid = float((counts * (counts - 1) * (B - counts)).sum())

    in_maps = []
    for b in range(N_CORES):
        cols = np.arange(R * b, R * b + R)
        livec = live_row[cols]
        row_cls = np.minimum(cls_of_row[cols], C - 1)
        row_m = counts[row_cls]
        # lhs extra chunk: onehot rows (anchor class), ones row, |a|^2 row
        ch = np.zeros((33, R), dtype=np.float16)
        ch[row_cls, np.arange(R)] = np.where(livec, OH, 0.0)
        ch[32, :] = np.float16(1.0)
        ch6 = np.zeros((33, R), dtype=np.float16)
        ch6[32, :] = sqa_all[cols].astype(np.float16)
        lhs_chunks = ePT[:, cols].reshape(K, 128, R)
        lhsT = np.ascontiguousarray(np.concatenate(
            [lhs_chunks[k] for k in range(K)], axis=1))
        rhsx = np.ascontiguousarray(
            np.concatenate([rhs_extra, ch, ch6], axis=1))

        ii = np.tile(np.arange(SLOT), spb)
        jj = np.arange(S)[None, :]
        valid = ((jj < row_m[:, None]) & (jj != ii[:, None])
                 & (ii[:, None] < row_m[:, None]))
        mb = np.where(valid, -SHIFT, -3032.0).astype(np.float16)
        in_maps.append({
            "lhsT": lhsT,
            "rhsa": rhsa,
            "rhsb": rhsb,
            "rhsx": rhsx,
            "mb": mb,
        })
    return S, R, C, in_maps, num_valid


def _combine(outs, num_valid, S, n_act=11, n_pool=0):
    n_dve = S - n_act - n_pool
    loss_sum = 0.0
    num_pos = 0.0
    R = outs[0].shape[0]
    for c in range(N_CORES):
        o = np.asarray(outs[c], dtype=np.float64)
        # DVE max-trick columns need the +NB*sum(pos) correction
        sums = (o[:, 0:2 * n_dve:2].sum() + NB * o[:, 2 * S].sum()
                + o[:, 2 * n_dve:2 * (n_dve + n_act):2].sum())
        cnts = o[:, 1:2 * n_dve:2].sum()
        sgn = o[:, 2 * n_dve + 1:2 * (n_dve + n_act):2].sum()
        cnts += 0.5 * sgn + 0.5 * NB * n_act * R
        loss_sum += sums
        num_pos += cnts
    loss = np.float32(loss_sum / (num_pos + 1e-5))
    frac = np.float32(num_pos / (num_valid + 1e-5))
    return (loss, frac)


def kernel(embeddings: np.ndarray, labels: np.ndarray):
    S, R, C, in_maps, num_valid = _prepare(embeddings, labels)
    key = (S, R, C)
    if key not in _cache:
        _cache[key] = _build(S, R, C)
    nc = _cache[key]
    res = run_bass_kernel_spmd(nc, in_maps, core_ids=list(range(N_CORES)))
    return _combine([res.results[c]["out"] for c in range(N_CORES)],
                    num_valid, S)


# revision 4
# speedup vs baseline: 2.0303x; 2.0303x over previous
"""BatchAllTripletLoss on 8 Trainium2 NeuronCores — padded class-slot grid.

Host layout:
  - Sort batch by label; pad each class to a slot of S = max_class_size
    columns (zero embeddings in pads). 32 classes x S=24 -> 768 padded
    anchor rows = 8 blocks x R=96 rows, one block per core; each core
    scores its 96 anchors against ALL 512 real negative columns.
  - Positives of anchor row r (slot k = r//S) are window cols [S*k, S*k+S)
    of the core's own padded cols -> band extraction is nslot FIXED
    rectangle ops (compile-time APs, SPMD-uniform, no DMA round trip).
  - The contraction gets an extra 34-row chunk: rows 0-31 carry
    1000*onehot(class) on both sides (gram accumulates 1e6 for same-class
    pairs = fused label mask, so sqrt input is never negative), row 32 is
    ones against |b|^2 (fused column-norm row), row 33 holds |a|^2 for the
    w-side row. num_valid is computed on host from label counts alone.

Device:
  - PE: grams (-2*A^T B + |b|^2 + 1e6*eq); anchor-norm bias via ACT/DVE.
  - n-side: dn = Sqrt(g + |a|^2) on ACT (f32), ndn = 32 - dn -> f16 (DVE
    tensor_scalar at 2x; the 32-shift keeps f16 error ~2e-3).
  - w-side: band2 = relu(g_w + |a|^2) per rectangle (DVE, from PSUM),
    dband = Sqrt (ACT), pos = dband + mb (f32; mb = -32 valid / -3032).
  - Hot loop over S slots x 2 single-src tensor_scalar ops (relu-sum
    accum + count accum) split DVE (fp16 4x) / ACT (Relu+Sign) / Pool.
  - No device reduction: raw accum columns are DMA'd out; host reduces.
"""
import sys
sys.path.insert(0, "/opt/trn_rl_repo")

import numpy as np
from contextlib import ExitStack

import concourse.bass as bass
import concourse.tile as tile
from concourse import bacc, mybir
from concourse.bass_utils import run_bass_kernel_spmd

F32 = mybir.dt.float32
F16 = mybir.dt.float16
Alu = mybir.AluOpType
Act = mybir.ActivationFunctionType
AX = mybir.AxisListType

B = 512
K = 4            # contraction chunks of 128 (512 dims)
KE = 34          # extra chunk rows: 32 onehot + ones + |a|^2
NB = 512         # negative columns per core
NH = 256         # half of NB
N_CORES = 8
SHIFT = 32.0
OH = 1000.0      # onehot amplitude -> 1e6 mask in the gram

_cache = {}

CNT_SCALE = 8192.0     # fused accum: acc = sum relu(x) + CNT_SCALE*count


def _register_fused_op():
    """Runtime-register a custom DVE op fusing relu-sum + count:
    acc = sum_n[ relu(in0+s0) + CNT_SCALE*(in0+s0 > s1) ]. Decode on host:
    count = floor(acc/CNT_SCALE) (valid since sum relu < CNT_SCALE)."""
    import concourse.dve_ops as dve_ops
    from concourse.dve_spec import Spec, Src0, C0, C1, C2, relu, lower, \
        _has_src1
    from concourse.dve_ops import DveOp
    from concourse.dve_uop import DveOpSpec
    name = "RELU_SUM_CNT_ANT"
    if name in dve_ops._SUB_OPCODE_FOR_NAME:
        for o in dve_ops.OPS:
            if o.name == name:
                return o
    spec = Spec(
        body=relu(Src0 + C0) + ((Src0 + C0) > C1) * C2,
        accum=__import__("operator").add,
        reference=lambda in0, s0, s1, imm2: (
            np.maximum(in0.astype(np.float32) + s0, 0.0)
            + imm2 * ((in0.astype(np.float32) + s0) > s1)),
    )
    op = DveOp(name, spec, subdim=False, uops_sha={})
    row = dve_ops._CUSTOM_DVE_ROW_BASE + len(dve_ops.OPS)
    assert row < 0x20
    dve_ops.OPS.append(op)
    dve_ops.CUSTOM_DVE_SPECS[name] = spec
    dve_ops._SUB_OPCODE_FOR_NAME[name] = row
    for ver in ("v3", "v4"):
        s = DveOpSpec(name=name, opcode=row, uops=lower(spec, ver=ver),
                      rd1_en=_has_src1(spec))
        object.__setattr__(op, "uops_sha", {**op.uops_sha, ver: s.sha(ver)})
    return op


def _build(S: int, R: int, C: int, n_act: int = 7, n_pool: int = 0,
           loop_iters: int | None = None):
    """S = band width (max class size), R = rows per core (32-aligned
    class slots), C = number of classes."""
    SLOT = 32 * ((S + 31) // 32)
    nslot = R // SLOT
    n_dve = S - n_act - n_pool
    assert C <= 32 and R % SLOT == 0

    nc = bacc.Bacc("TRN2", target_bir_lowering=False, debug=False,
                   num_devices=N_CORES)

    lhsT_d = nc.dram_tensor("lhsT", [128, K * R], F16,
                            kind="ExternalInput")
    rhsa_d = nc.dram_tensor("rhsa", [128, K * NH], F16,
                            kind="ExternalInput")
    rhsb_d = nc.dram_tensor("rhsb", [128, K * NH], F16,
                            kind="ExternalInput")
    rhsx_d = nc.dram_tensor("rhsx", [33, NB + 2 * R], F16,
                            kind="ExternalInput")
    mb_d = nc.dram_tensor("mb", [R, S], F16, kind="ExternalInput")
    out_d = nc.dram_tensor("out", [R, S + n_act], F32,
                           kind="ExternalOutput")

    with tile.TileContext(nc) as tc, ExitStack() as ctx:
        pool = ctx.enter_context(tc.tile_pool(name="sbuf", bufs=2))
        spool = ctx.enter_context(tc.tile_pool(name="scr", bufs=3))
        ppool = ctx.enter_context(tc.tile_pool(name="psum", bufs=1, space="PSUM"))

        def _body():
            # ---- input DMAs ----
            lhsT_t = pool.tile([128, K * R], F16)
            rhsa_t = pool.tile([128, K, NH], F16)
            rhsb_t = pool.tile([128, K, NH], F16)
            rhsx_t = pool.tile([33, NB + 2 * R], F16)
            mb_t = pool.tile([R, S], F16)
            nc.sync.dma_start(lhsT_t[:], lhsT_d.ap())
            nc.sync.dma_start(rhsa_t[:], rhsa_d.ap())
            nc.scalar.dma_start(rhsb_t[:], rhsb_d.ap())
            lhsT = [lhsT_t[:, k * R:(k + 1) * R] for k in range(K)]
            rhs = {0: [rhsa_t[:, k, :] for k in range(K)],
                   1: [rhsb_t[:, k, :] for k in range(K)]}
            lhs5 = rhsx_t[0:33, NB:NB + R]
            lhs5_ones = rhsx_t[32:33, NB:NB + R]
            lhs6_nrm = rhsx_t[32:33, NB + R:NB + 2 * R]
            rhs5 = {0: rhsx_t[0:33, 0:NH], 1: rhsx_t[0:33, NH:NB]}

            # ---- PE warmup: ramp the tensor engine out of low p-state ----
            nc.scalar.dma_start(rhsx_t[:], rhsx_d.ap())
            wsrc = pool.tile([128, NB], F16)
            nc.gpsimd.memset(wsrc[:], 0.0)
            nc.sync.dma_start(mb_t[:], mb_d.ap())
            dumt = pool.tile([1, 2], F32)
            nc.vector.memset(dumt[:], 1.0)
            one16 = pool.tile([33, 1], F16)
            nc.vector.memset(one16[:], 1.0)
            # pin the ACT table set (sqrt_and_others has all our funcs)
            dums = pool.tile([1, 2], F32)
            nc.scalar.activation(dums[:], dumt[:], Act.Sqrt)
            warm = ppool.tile([1, NB], F32, tag="warm", name="warm")
            for w in range(7):
                nc.tensor.matmul(warm[:], wsrc[:, 0:1], wsrc[:],
                                 start=True, stop=True)

            # ---- anchor-norm column: |a|^2 row as weights x ones column ----
            nrma = ppool.tile([R, 1], F32, tag="nrma", name="nrma")
            nc.tensor.matmul(nrma[:], lhs6_nrm, one16[32:33, :],
                             start=True, stop=True)
            nrma_s = pool.tile([R, 1], F32)
            nc.vector.tensor_copy(nrma_s[:], nrma[:])

            # ---- lhs scaled (-2x) ----
            lm2 = [pool.tile([128, R], F16, tag=f"lm2{k}", name=f"lm2{k}")
                   for k in range(K)]
            for k in range(K):
                nc.vector.tensor_scalar_mul(lm2[k][:], lhsT[k], -2.0)

            # ---- w-side gram + fused-relu band rectangles ----
            g_w = ppool.tile([R, R], F32, tag="gw", name="gw")
            for k in range(K):
                nc.tensor.matmul(g_w[:], lm2[k][:], lhsT[k],
                                 start=(k == 0), stop=False)
            nc.tensor.matmul(g_w[:], lhs5_ones, lhs6_nrm,
                             start=False, stop=True)
            band2 = pool.tile([R, S], F32)
            for k in range(nslot):
                nc.vector.tensor_scalar(
                    out=band2[k * SLOT:(k + 1) * SLOT, :],
                    in0=g_w[k * SLOT:(k + 1) * SLOT,
                            k * SLOT:k * SLOT + S],
                    scalar1=nrma_s[k * SLOT:(k + 1) * SLOT, :], scalar2=0.0,
                    op0=Alu.add, op1=Alu.max)
            dband = pool.tile([R, S], F32)
            nc.scalar.activation(dband[:], band2[:], Act.Sqrt)
            pos = pool.tile([R, S], F32)
            nc.vector.tensor_tensor(out=pos[:], in0=dband[:], in1=mb_t[:],
                                    op=Alu.add)


            # ---- n-side grams (onehot mask + col norms fused) ----
            ndn = pool.tile([R, NB], F16)
            for h in (0, 1):
                g = ppool.tile([R, NH], F32, tag=f"gn{h}", name=f"gn{h}")
                for k in range(K):
                    nc.tensor.matmul(g[:], lm2[k][:], rhs[h][k],
                                     start=(k == 0), stop=False)
                nc.tensor.matmul(g[:], lhs5, rhs5[h],
                                 start=False, stop=True)
                dn = spool.tile([R, NH], F32, tag=f"dn{h}", name=f"dn{h}")
                nc.scalar.activation(dn[:], g[:], Act.Sqrt,
                                     bias=nrma_s[:], scale=1.0)
                nc.vector.tensor_scalar(
                    out=ndn[:, h * NH:(h + 1) * NH], in0=dn[:],
                    scalar1=-1.0, scalar2=SHIFT,
                    op0=Alu.mult, op1=Alu.add)

            # ---- hot loop ----
            # DVE slots j < n_dve: ONE fused custom-DVE instruction each:
            #   acc = sum relu(ndn+p) + CNT_SCALE*count   (host-decoded)
            # ACT slots (the last n_act): native Relu/Sign accumulation.
            fop = _register_fused_op()
            out_t = pool.tile([R, n_dve + 2 * n_act], F32)
            acc_a = pool.tile([R, 2 * max(n_act, 1)], F32)

            jd = ja = 0
            for j in range(S):
                pj = pos[:, j:j + 1]
                if j >= n_dve:
                    s1 = ppool.tile([R, NB], F32, tag="asc",
                                    name=f"asc1_{j}", bufs=2)
                    nc.scalar.activation(s1[:], ndn[:], Act.Relu,
                                         bias=pj, scale=1.0,
                                         accum_out=acc_a[:, 2 * ja:2 * ja + 1])
                    s2 = ppool.tile([R, NB], F32, tag="asc",
                                    name=f"asc2_{j}", bufs=2)
                    nc.scalar.activation(s2[:], ndn[:], Act.Sign,
                                         bias=pj, scale=1.0,
                                         accum_out=acc_a[:, 2 * ja + 1:2 * ja + 2])
                    ja += 1
                else:
                    s1 = spool.tile([R, NB], F16, tag="dsc", name=f"dsc1_{j}")
                    nc.vector._custom_dve(
                        fop, out=s1[:], in0=ndn[:], s0=pj, s1=0.0,
                        imm2=CNT_SCALE, accum_out=out_t[:, jd:jd + 1])
                    jd += 1
            assert ja == n_act and jd == n_dve

            if n_act:
                nc.vector.tensor_copy(
                    out_t[:, n_dve:n_dve + 2 * n_act], acc_a[:])

            nc.sync.dma_start(out_d.ap(), out_t[:])

        if loop_iters is None:
            _body()
        else:
            with tc.For_i(0, loop_iters, 1):
                _body()

    nc.compile()
    return nc


def _ilv(a, nchunk):
    """[nchunk*128 (contraction), x] -> [128, nchunk*x] chunk-interleaved."""
    x = a.shape[1]
    return np.ascontiguousarray(
        a.reshape(nchunk, 128, x).transpose(1, 0, 2).reshape(128, nchunk * x))


def _prepare(embeddings: np.ndarray, labels: np.ndarray):
    emb = np.ascontiguousarray(np.asarray(embeddings, dtype=np.float32))
    lab = np.asarray(labels)

    perm = np.argsort(lab, kind="stable")
    e_s = emb[perm]
    lab_s = lab[perm]
    classes, starts, counts = np.unique(lab_s, return_index=True,
                                        return_counts=True)
    C = len(classes)
    S = int(counts.max())
    SLOT = 32 * ((S + 31) // 32)
    spb = -(-C // N_CORES)            # class slots per block
    R = spb * SLOT
    assert R <= 128, f"padded rows per core {R} > 128"

    cls_of_col = np.searchsorted(starts, np.arange(B), side="right") - 1

    # padded anchors: class c -> slot c, rows [SLOT*c, SLOT*c+m_c)
    nP = N_CORES * R
    eP = np.zeros((nP, B), dtype=np.float32)
    cls_of_row = np.repeat(np.arange(-(-nP // SLOT)), SLOT)[:nP]
    live_row = np.zeros((nP,), dtype=bool)
    for c in range(C):
        eP[SLOT * c:SLOT * c + counts[c]] = \
            e_s[starts[c]:starts[c] + counts[c]]
        live_row[SLOT * c:SLOT * c + counts[c]] = True
    ePT = np.ascontiguousarray(eP.T).astype(np.float16)
    e_sT = np.ascontiguousarray(e_s.T).astype(np.float16)  # [512, 512]

    sqa_all = (ePT.astype(np.float32) ** 2).sum(0)         # [768]
    sqn = (e_sT.astype(np.float32) ** 2).sum(0)            # [512]

    # rhs extra chunks: onehot rows + |b|^2 in row 32
    def rhs_chunk(cols):
        ch = np.zeros((128, len(cols)), dtype=np.float16)
        ch[cls_of_col[cols], np.arange(len(cols))] = OH
        ch[32, :] = sqn[cols].astype(np.float16)
        return ch

    rhsa = _ilv(e_sT[:, :NH], K)
    rhsb = _ilv(e_sT[:, NH:], K)
    rhs_extra = np.ascontiguousarray(rhs_chunk(np.arange(NB))[0:33])

    num_valid = float((counts * (counts - 1) * (B - counts)).sum())

    in_maps = []
    for b in range(N_CORES):
        cols = np.arange(R * b, R * b + R)
        livec = live_row[cols]
        row_cls = np.minimum(cls_of_row[cols], C - 1)
        row_m = counts[row_cls]
        # lhs extra chunk: onehot rows (anchor class), ones row, |a|^2 row
        ch = np.zeros((33, R), dtype=np.float16)
        ch[row_cls, np.arange(R)] = np.where(livec, OH, 0.0)
        ch[32, :] = np.float16(1.0)
        ch6 = np.zeros((33, R), dtype=np.float16)
        ch6[32, :] = sqa_all[cols].astype(np.float16)
        lhs_chunks = ePT[:, cols].reshape(K, 128, R)
        lhsT = np.ascontiguousarray(np.concatenate(
            [lhs_chunks[k] for k in range(K)], axis=1))
        rhsx = np.ascontiguousarray(
            np.concatenate([rhs_extra, ch, ch6], axis=1))

        ii = np.tile(np.arange(SLOT), spb)
        jj = np.arange(S)[None, :]
        valid = ((jj < row_m[:, None]) & (jj != ii[:, None])
                 & (ii[:, None] < row_m[:, None]))
        mb = np.where(valid, -SHIFT, -3032.0).astype(np.float16)
        in_maps.append({
            "lhsT": lhsT,
            "rhsa": rhsa,
            "rhsb": rhsb,
            "rhsx": rhsx,
            "mb": mb,
        })
    return S, R, C, in_maps, num_valid


def _combine(outs, num_valid, S, n_act=7, n_pool=0):
    n_dve = S - n_act - n_pool
    loss_sum = 0.0
    num_pos = 0.0
    R = outs[0].shape[0]
    for c in range(N_CORES):
        o = np.asarray(outs[c], dtype=np.float64)
        # fused DVE columns: count = floor(acc/CNT_SCALE), relu = rest
        fused = o[:, 0:n_dve]
        fcnt = np.floor(fused / CNT_SCALE)
        sums = ((fused - CNT_SCALE * fcnt).sum()
                + o[:, n_dve:n_dve + 2 * n_act:2].sum())
        cnts = fcnt.sum()
        sgn = o[:, n_dve + 1:n_dve + 2 * n_act:2].sum()
        cnts += 0.5 * sgn + 0.5 * NB * n_act * R
        loss_sum += sums
        num_pos += cnts
    loss = np.float32(loss_sum / (num_pos + 1e-5))
    frac = np.float32(num_pos / (num_valid + 1e-5))
    return (loss, frac)


def kernel(embeddings: np.ndarray, labels: np.ndarray):
    S, R, C, in_maps, num_valid = _prepare(embeddings, labels)
    key = (S, R, C)
    if key not in _cache:
        _cache[key] = _build(S, R, C)
    nc = _cache[key]
    res = run_bass_kernel_spmd(nc, in_maps, core_ids=list(range(N_CORES)))
    return _combine([res.results[c]["out"] for c in range(N_CORES)],
                    num_valid, S)


# revision 5
# speedup vs baseline: 2.5184x; 1.2404x over previous
"""BatchAllTripletLoss on 8 Trainium2 NeuronCores — padded class-slot grid.

Host layout:
  - Sort batch by label; pad each class to a 32-row slot (zero embeddings
    in pads; engine ops need 32-aligned partition bases). 32 classes ->
    8 blocks x R=128 rows, one block per core; each core scores its block
    against ALL 512 real negative columns.
  - Positives of anchor row r (slot k = r//32) are window cols
    [32k, 32k+S) of the core's own padded cols -> band extraction is 4
    FIXED rectangle ops (compile-time APs, SPMD-uniform, no DMA trip).
  - The contraction gets an extra 34-row chunk: rows 0-31 carry
    1000*onehot(class) on both sides (gram accumulates 1e6 for same-class
    pairs = fused label mask, so sqrt input is never negative), row 32 is
    ones against |b|^2 (fused column-norm row), row 33 holds |a|^2 for the
    w-side row. num_valid is computed on host from label counts alone.

Device:
  - PE: grams (-2*A^T B + |b|^2 + 1e6*eq); anchor-norm bias via ACT/DVE.
  - n-side: dn = Sqrt(g + |a|^2) on ACT (f32), ndn = 32 - dn -> f16 (DVE
    tensor_scalar at 2x; the 32-shift keeps f16 error ~2e-3).
  - w-side: band2 = relu(g_w + |a|^2) per rectangle (DVE, from PSUM),
    dband = Sqrt (ACT), pos = dband + mb (f32; mb = -32 valid / -3032).
  - Hot loop over S=24 band slots split DVE/ACT by measured instr cost:
    DVE slots use ONE runtime-registered custom DVE op per slot
    (acc = sum relu(ndn+p) + 8192*count, host floor-decoded); ACT slots
    use native Relu/Sign accumulation (accum-read costs 187ns each).
  - No device reduction: raw accum columns are DMA'd out; host reduces.
"""
import sys
sys.path.insert(0, "/opt/trn_rl_repo")

import numpy as np
from contextlib import ExitStack

import concourse.bass as bass
import concourse.tile as tile
from concourse import bacc, mybir
from concourse.bass_utils import run_bass_kernel_spmd

F32 = mybir.dt.float32
F16 = mybir.dt.float16
Alu = mybir.AluOpType
Act = mybir.ActivationFunctionType
AX = mybir.AxisListType

B = 512
K = 4            # contraction chunks of 128 (512 dims)
KE = 34          # extra chunk rows: 32 onehot + ones + |a|^2
NB = 512         # negative columns per core
NH = 256         # half of NB
N_CORES = 8
SHIFT = 32.0
OH = 1000.0      # onehot amplitude -> 1e6 mask in the gram

_cache = {}

CNT_SCALE = 8192.0     # fused accum: acc = sum relu(x) + CNT_SCALE*count


def _register_fused_op():
    """Runtime-register a custom DVE op fusing relu-sum + count:
    acc = sum_n[ relu(in0+s0) + CNT_SCALE*(in0+s0 > s1) ]. Decode on host:
    count = floor(acc/CNT_SCALE) (valid since sum relu < CNT_SCALE)."""
    import concourse.dve_ops as dve_ops
    from concourse.dve_spec import Spec, Src0, C0, C1, C2, relu, lower, \
        _has_src1
    from concourse.dve_ops import DveOp
    from concourse.dve_uop import DveOpSpec
    name = "RELU_SUM_CNT_ANT"
    if name in dve_ops._SUB_OPCODE_FOR_NAME:
        for o in dve_ops.OPS:
            if o.name == name:
                return o
    spec = Spec(
        body=relu(Src0 + C0) + ((Src0 + C0) > C1) * C2,
        accum=__import__("operator").add,
        reference=lambda in0, s0, s1, imm2: (
            np.maximum(in0.astype(np.float32) + s0, 0.0)
            + imm2 * ((in0.astype(np.float32) + s0) > s1)),
    )
    op = DveOp(name, spec, subdim=False, uops_sha={})
    row = dve_ops._CUSTOM_DVE_ROW_BASE + len(dve_ops.OPS)
    assert row < 0x20
    dve_ops.OPS.append(op)
    dve_ops.CUSTOM_DVE_SPECS[name] = spec
    dve_ops._SUB_OPCODE_FOR_NAME[name] = row
    for ver in ("v3", "v4"):
        s = DveOpSpec(name=name, opcode=row, uops=lower(spec, ver=ver),
                      rd1_en=_has_src1(spec))
        object.__setattr__(op, "uops_sha", {**op.uops_sha, ver: s.sha(ver)})
    return op


def _build(S: int, R: int, C: int, n_act: int = 7, n_pool: int = 0,
           loop_iters: int | None = None):
    """S = band width (max class size), R = rows per core (32-aligned
    class slots), C = number of classes."""
    SLOT = 32 * ((S + 31) // 32)
    nslot = R // SLOT
    n_dve = S - n_act - n_pool
    assert C <= 32 and R % SLOT == 0

    nc = bacc.Bacc("TRN2", target_bir_lowering=False, debug=False,
                   num_devices=N_CORES)

    lhsT_d = nc.dram_tensor("lhsT", [128, K * R], F16,
                            kind="ExternalInput")
    rhsa_d = nc.dram_tensor("rhsa", [128, K * NH], F16,
                            kind="ExternalInput")
    rhsb_d = nc.dram_tensor("rhsb", [128, K * NH], F16,
                            kind="ExternalInput")
    rhsx_d = nc.dram_tensor("rhsx", [33, NB + 2 * R], F16,
                            kind="ExternalInput")
    mb_d = nc.dram_tensor("mb", [R, S], F16, kind="ExternalInput")
    out_d = nc.dram_tensor("out", [R, S + n_act], F32,
                           kind="ExternalOutput")

    with tile.TileContext(nc) as tc, ExitStack() as ctx:
        pool = ctx.enter_context(tc.tile_pool(name="sbuf", bufs=2))
        spool = ctx.enter_context(tc.tile_pool(name="scr", bufs=3))
        ppool = ctx.enter_context(tc.tile_pool(name="psum", bufs=1, space="PSUM"))

        def _body():
            # ---- input DMAs ----
            lhsT_t = pool.tile([128, K * R], F16)
            rhsa_t = pool.tile([128, K, NH], F16)
            rhsb_t = pool.tile([128, K, NH], F16)
            rhsx_t = pool.tile([33, NB + 2 * R], F16)
            mb_t = pool.tile([R, S], F16)
            nc.sync.dma_start(lhsT_t[:], lhsT_d.ap())
            nc.sync.dma_start(rhsa_t[:], rhsa_d.ap())
            nc.scalar.dma_start(rhsb_t[:], rhsb_d.ap())
            lhsT = [lhsT_t[:, k * R:(k + 1) * R] for k in range(K)]
            rhs = {0: [rhsa_t[:, k, :] for k in range(K)],
                   1: [rhsb_t[:, k, :] for k in range(K)]}
            lhs5 = rhsx_t[0:33, NB:NB + R]
            lhs5_ones = rhsx_t[32:33, NB:NB + R]
            lhs6_nrm = rhsx_t[32:33, NB + R:NB + 2 * R]
            rhs5 = {0: rhsx_t[0:33, 0:NH], 1: rhsx_t[0:33, NH:NB]}

            # ---- PE warmup: ramp the tensor engine out of low p-state ----
            nc.scalar.dma_start(rhsx_t[:], rhsx_d.ap())
            wsrc = pool.tile([128, NB], F16)
            nc.gpsimd.memset(wsrc[:], 0.0)
            nc.sync.dma_start(mb_t[:], mb_d.ap())
            dumt = pool.tile([1, 2], F32)
            nc.vector.memset(dumt[:], 1.0)
            one16 = pool.tile([33, 1], F16)
            nc.vector.memset(one16[:], 1.0)
            # pin the ACT table set (sqrt_and_others has all our funcs)
            dums = pool.tile([1, 2], F32)
            nc.scalar.activation(dums[:], dumt[:], Act.Sqrt)
            warm = ppool.tile([1, NB], F32, tag="warm", name="warm")
            for w in range(7):
                nc.tensor.matmul(warm[:], wsrc[:, 0:1], wsrc[:],
                                 start=True, stop=True)

            # ---- anchor-norm column: |a|^2 row as weights x ones column ----
            nrma = ppool.tile([R, 1], F32, tag="nrma", name="nrma")
            nc.tensor.matmul(nrma[:], lhs6_nrm, one16[32:33, :],
                             start=True, stop=True)
            nrma_s = pool.tile([R, 1], F32)
            nc.vector.tensor_copy(nrma_s[:], nrma[:])

            # ---- lhs scaled (-2x) ----
            lm2 = [pool.tile([128, R], F16, tag=f"lm2{k}", name=f"lm2{k}")
                   for k in range(K)]
            for k in range(K):
                nc.vector.tensor_scalar_mul(lm2[k][:], lhsT[k], -2.0)

            # ---- w-side gram + fused-relu band rectangles ----
            g_w = ppool.tile([R, R], F32, tag="gw", name="gw")
            for k in range(K):
                nc.tensor.matmul(g_w[:], lm2[k][:], lhsT[k],
                                 start=(k == 0), stop=False)
            nc.tensor.matmul(g_w[:], lhs5_ones, lhs6_nrm,
                             start=False, stop=True)
            band2 = pool.tile([R, S], F32)
            for k in range(nslot):
                nc.vector.tensor_scalar(
                    out=band2[k * SLOT:(k + 1) * SLOT, :],
                    in0=g_w[k * SLOT:(k + 1) * SLOT,
                            k * SLOT:k * SLOT + S],
                    scalar1=nrma_s[k * SLOT:(k + 1) * SLOT, :], scalar2=0.0,
                    op0=Alu.add, op1=Alu.max)
            dband = pool.tile([R, S], F32)
            nc.scalar.activation(dband[:], band2[:], Act.Sqrt)
            pos = pool.tile([R, S], F32)
            nc.vector.tensor_tensor(out=pos[:], in0=dband[:], in1=mb_t[:],
                                    op=Alu.add)


            # ---- n-side grams (onehot mask + col norms fused) ----
            ndn = pool.tile([R, NB], F16)
            for h in (0, 1):
                g = ppool.tile([R, NH], F32, tag=f"gn{h}", name=f"gn{h}")
                for k in range(K):
                    nc.tensor.matmul(g[:], lm2[k][:], rhs[h][k],
                                     start=(k == 0), stop=False)
                nc.tensor.matmul(g[:], lhs5, rhs5[h],
                                 start=False, stop=True)
                dn = spool.tile([R, NH], F32, tag=f"dn{h}", name=f"dn{h}")
                nc.scalar.activation(dn[:], g[:], Act.Sqrt,
                                     bias=nrma_s[:], scale=1.0)
                nc.vector.tensor_scalar(
                    out=ndn[:, h * NH:(h + 1) * NH], in0=dn[:],
                    scalar1=-1.0, scalar2=SHIFT,
                    op0=Alu.mult, op1=Alu.add)

            # ---- hot loop ----
            # DVE slots j < n_dve: ONE fused custom-DVE instruction each:
            #   acc = sum relu(ndn+p) + CNT_SCALE*count   (host-decoded)
            # ACT slots (the last n_act): native Relu/Sign accumulation.
            fop = _register_fused_op()
            out_t = pool.tile([R, n_dve + 2 * n_act], F32)
            acc_a = pool.tile([R, 2 * max(n_act, 1)], F32)

            jd = ja = 0
            for j in range(S):
                pj = pos[:, j:j + 1]
                if j >= n_dve:
                    s1 = ppool.tile([R, NB], F32, tag="asc",
                                    name=f"asc1_{j}", bufs=2)
                    nc.scalar.activation(s1[:], ndn[:], Act.Relu,
                                         bias=pj, scale=1.0,
                                         accum_out=acc_a[:, 2 * ja:2 * ja + 1])
                    s2 = ppool.tile([R, NB], F32, tag="asc",
                                    name=f"asc2_{j}", bufs=2)
                    nc.scalar.activation(s2[:], ndn[:], Act.Sign,
                                         bias=pj, scale=1.0,
                                         accum_out=acc_a[:, 2 * ja + 1:2 * ja + 2])
                    ja += 1
                else:
                    s1 = spool.tile([R, NB], F16, tag="dsc", name=f"dsc1_{j}")
                    nc.vector._custom_dve(
                        fop, out=s1[:], in0=ndn[:], s0=pj, s1=0.0,
                        imm2=CNT_SCALE, accum_out=out_t[:, jd:jd + 1])
                    jd += 1
            assert ja == n_act and jd == n_dve

            if n_act:
                nc.vector.tensor_copy(
                    out_t[:, n_dve:n_dve + 2 * n_act], acc_a[:])

            nc.sync.dma_start(out_d.ap(), out_t[:])

        if loop_iters is None:
            _body()
        else:
            with tc.For_i(0, loop_iters, 1):
                _body()

    nc.compile()
    return nc


def _ilv(a, nchunk):
    """[nchunk*128 (contraction), x] -> [128, nchunk*x] chunk-interleaved."""
    x = a.shape[1]
    return np.ascontiguousarray(
        a.reshape(nchunk, 128, x).transpose(1, 0, 2).reshape(128, nchunk * x))


def _prepare(embeddings: np.ndarray, labels: np.ndarray):
    emb = np.ascontiguousarray(np.asarray(embeddings, dtype=np.float32))
    lab = np.asarray(labels)

    perm = np.argsort(lab, kind="stable")
    e_s = emb[perm]
    lab_s = lab[perm]
    classes, starts, counts = np.unique(lab_s, return_index=True,
                                        return_counts=True)
    C = len(classes)
    S = int(counts.max())
    SLOT = 32 * ((S + 31) // 32)
    spb = -(-C // N_CORES)            # class slots per block
    R = spb * SLOT
    assert R <= 128, f"padded rows per core {R} > 128"

    cls_of_col = np.searchsorted(starts, np.arange(B), side="right") - 1

    # padded anchors: class c -> slot c, rows [SLOT*c, SLOT*c+m_c)
    nP = N_CORES * R
    eP = np.zeros((nP, B), dtype=np.float32)
    cls_of_row = np.repeat(np.arange(-(-nP // SLOT)), SLOT)[:nP]
    live_row = np.zeros((nP,), dtype=bool)
    for c in range(C):
        eP[SLOT * c:SLOT * c + counts[c]] = \
            e_s[starts[c]:starts[c] + counts[c]]
        live_row[SLOT * c:SLOT * c + counts[c]] = True
    ePT = np.ascontiguousarray(eP.T).astype(np.float16)
    e_sT = np.ascontiguousarray(e_s.T).astype(np.float16)  # [512, 512]

    sqa_all = (ePT.astype(np.float32) ** 2).sum(0)         # [768]
    sqn = (e_sT.astype(np.float32) ** 2).sum(0)            # [512]

    # rhs extra chunks: onehot rows + |b|^2 in row 32
    def rhs_chunk(cols):
        ch = np.zeros((128, len(cols)), dtype=np.float16)
        ch[cls_of_col[cols], np.arange(len(cols))] = OH
        ch[32, :] = sqn[cols].astype(np.float16)
        return ch

    rhsa = _ilv(e_sT[:, :NH], K)
    rhsb = _ilv(e_sT[:, NH:], K)
    rhs_extra = np.ascontiguousarray(rhs_chunk(np.arange(NB))[0:33])

    num_valid = float((counts * (counts - 1) * (B - counts)).sum())

    in_maps = []
    for b in range(N_CORES):
        cols = np.arange(R * b, R * b + R)
        livec = live_row[cols]
        row_cls = np.minimum(cls_of_row[cols], C - 1)
        row_m = counts[row_cls]
        # lhs extra chunk: onehot rows (anchor class), ones row, |a|^2 row
        ch = np.zeros((33, R), dtype=np.float16)
        ch[row_cls, np.arange(R)] = np.where(livec, OH, 0.0)
        ch[32, :] = np.float16(1.0)
        ch6 = np.zeros((33, R), dtype=np.float16)
        ch6[32, :] = sqa_all[cols].astype(np.float16)
        lhs_chunks = ePT[:, cols].reshape(K, 128, R)
        lhsT = np.ascontiguousarray(np.concatenate(
            [lhs_chunks[k] for k in range(K)], axis=1))
        rhsx = np.ascontiguousarray(
            np.concatenate([rhs_extra, ch, ch6], axis=1))

        ii = np.tile(np.arange(SLOT), spb)
        jj = np.arange(S)[None, :]
        valid = ((jj < row_m[:, None]) & (jj != ii[:, None])
                 & (ii[:, None] < row_m[:, None]))
        mb = np.where(valid, -SHIFT, -3032.0).astype(np.float16)
        in_maps.append({
            "lhsT": lhsT,
            "rhsa": rhsa,
            "rhsb": rhsb,
            "rhsx": rhsx,
            "mb": mb,
        })
    return S, R, C, in_maps, num_valid


def _combine(outs, num_valid, S, n_act=7, n_pool=0):
    n_dve = S - n_act - n_pool
    loss_sum = 0.0
    num_pos = 0.0
    R = outs[0].shape[0]
    for c in range(N_CORES):
        o = np.asarray(outs[c], dtype=np.float64)
        # fused DVE columns: count = floor(acc/CNT_SCALE), relu = rest
        fused = o[:, 0:n_dve]
        fcnt = np.floor(fused / CNT_SCALE)
        sums = ((fused - CNT_SCALE * fcnt).sum()
                + o[:, n_dve:n_dve + 2 * n_act:2].sum())
        cnts = fcnt.sum()
        sgn = o[:, n_dve + 1:n_dve + 2 * n_act:2].sum()
        cnts += 0.5 * sgn + 0.5 * NB * n_act * R
        loss_sum += sums
        num_pos += cnts
    loss = np.float32(loss_sum / (num_pos + 1e-5))
    frac = np.float32(num_pos / (num_valid + 1e-5))
    return (loss, frac)


def kernel(embeddings: np.ndarray, labels: np.ndarray):
    S, R, C, in_maps, num_valid = _prepare(embeddings, labels)
    key = (S, R, C)
    if key not in _cache:
        _cache[key] = _build(S, R, C)
    nc = _cache[key]
    res = run_bass_kernel_spmd(nc, in_maps, core_ids=list(range(N_CORES)))
    return _combine([res.results[c]["out"] for c in range(N_CORES)],
                    num_valid, S)
